# revision 6
# baseline (speedup 1.0000x reference)
"""Trainium2 Bass kernel v2 for nn_GAT_79224966742097 — mega-loop design.

Backend empirical law: every top-level instruction costs ~50us to dispatch
regardless of size; instructions inside a For_i loop are only taxed for the
first ~210 dynamic executions (~11 ms total), then nearly free.  So the
whole two-layer GAT is ONE For_i loop of 25 iterations:

  i in [0,16): layer0 attention m-tile i   (accumulates U0 for real)
  every iter : h'0 projection (idempotent recompute)
  every iter : glue = U0/denom -> gelu -> instance-norm1 -> h'1
               (garbage before i=16, correct+idempotent from i=16)
  i in [17,25): layer1 attention m-pair (i+7)%8  (accumulates U1)

Accumulator writes are steered between the real accumulator and a scratch
sink via register arithmetic: off = scratch - sel*(scratch - real).

Precision: the layer0 -> x1 chain is all f32/f32r (layer1's exp amplifies
x1 errors ~3x, so bf16 anywhere upstream fails the 2e-2 gate); only
layer1's E/hpr operands are bf16.  Softmax denominators come from an
all-ones 65th lhsT column; 1/denom is broadcast across partitions via a
DRAM bounce (f32).  m-dependent stationary operands are DMA-staged into
fixed tiles each iteration (ldweights cannot take register offsets).
"""

import numpy as np

import concourse.bass as bass
import concourse.bacc as bacc
import concourse.mybir as mybir
import concourse.tile as tile
from contextlib import ExitStack
from concourse.masks import make_identity

F32 = mybir.dt.float32
F32R = mybir.dt.float32r
BF16 = mybir.dt.bfloat16
AX = mybir.AluOpType
AF = mybir.ActivationFunctionType
ds = bass.ds

N = 2048
EPS = 1e-5
NEG_SLOPE = 0.2
N_CORES = 8
N_ITER = 26
DEBUG = False


def build_bass(sim_safe=False, repeat=1):
    # sim_safe accepted for interface compatibility; unused.
    nc = bacc.Bacc("TRN2", debug=False)

    x_d = nc.dram_tensor("x", [N, 64], F32, kind="ExternalInput")
    a0_d = nc.dram_tensor("a0", [128, 2, N], F32, kind="ExternalInput")
    w0_d = nc.dram_tensor("w0", [64, 2, 128], F32, kind="ExternalInput")
    b0_d = nc.dram_tensor("b0", [64, 1], F32, kind="ExternalInput")
    a1_d = nc.dram_tensor("a1", [128, N], F32, kind="ExternalInput")
    w1_d = nc.dram_tensor("w1", [64, 4, 128], F32, kind="ExternalInput")
    out_d = nc.dram_tensor("out", [N, 64], F32, kind="ExternalOutput")
    dbg = {}
    if DEBUG:
        dbg['hpT'] = nc.dram_tensor("dbg_hpT", [128, 2, N], F32,
                                    kind="ExternalOutput")
        dbg['E0'] = nc.dram_tensor("dbg_E0", [128, 4, N], F32,
                                   kind="ExternalOutput")
        dbg['u0a'] = nc.dram_tensor("dbg_u0a", [65, 4 * N + 1024], F32,
                                    kind="ExternalOutput")
        dbg['x1T'] = nc.dram_tensor("dbg_x1T", [64, 4, N], F32,
                                    kind="ExternalOutput")
        dbg['hpT1'] = nc.dram_tensor("dbg_hpT1", [128, N], F32,
                                     kind="ExternalOutput")
        dbg['E1'] = nc.dram_tensor("dbg_E1", [128, 2, N], BF16,
                                   kind="ExternalOutput")
        dbg['u1a'] = nc.dram_tensor("dbg_u1a", [65, N + 1024], F32,
                                    kind="ExternalOutput")

    with tile.TileContext(nc) as tc, ExitStack() as ctx:
        const = ctx.enter_context(tc.tile_pool(name="const", bufs=1))
        sb = ctx.enter_context(tc.tile_pool(name="sb", bufs=1))
        ps = ctx.enter_context(tc.tile_pool(name="ps", bufs=1, space="PSUM"))
        dram = ctx.enter_context(tc.tile_pool(name="dram", bufs=1,
                                              space="DRAM"))
        for _rep in range(repeat):
            body(nc, tc, const, sb, ps, dram,
                 x_d, a0_d, w0_d, b0_d, a1_d, w1_d, out_d, dbg)
    nc.compile()
    return nc


def body(nc, tc, const, sb, ps, dram, x_d, a0_d, w0_d, b0_d, a1_d, w1_d,
         out_d, dbg):
    # ---------------- static prologue ----------------
    ident = const.tile([128, 128], F32, name="ident", uniquify=True)
    make_identity(nc, ident)

    eps_sb = const.tile([64, 1], F32, tag="eps")
    nc.vector.memset(eps_sb, EPS)

    b0_sb = const.tile([64, 1], F32, tag="b0")
    nc.sync.dma_start(out=b0_sb, in_=b0_d.ap())
    w0_sb = const.tile([64, 2, 128], F32, tag="w0")
    nc.sync.dma_start(out=w0_sb.bitcast(F32R), in_=w0_d.ap().bitcast(F32R))
    w1_sb = const.tile([64, 4, 128], F32, tag="w1")
    nc.sync.dma_start(out=w1_sb.bitcast(F32R), in_=w1_d.ap().bitcast(F32R))

    # x loaded transposed, instance-norm0 applied statically in place
    xT = sb.tile([64, N], F32, tag="xT")
    nc.sync.dma_start(out=xT.bitcast(F32R),
                      in_=x_d.ap().rearrange("n f -> f n").bitcast(F32R))
    st0 = sb.tile([64, 4, 6], F32, tag="st0")
    for c in range(4):
        nc.vector.bn_stats(out=st0[:, c, :], in_=xT[:, c * 512:(c + 1) * 512])
    mv0 = sb.tile([64, 2], F32, tag="mv0")
    nc.vector.bn_aggr(out=mv0, in_=st0)
    sd0 = sb.tile([64, 1], F32, tag="sd0")
    nc.scalar.activation(out=sd0, in_=mv0[:, 1:2], func=AF.Sqrt, bias=eps_sb)
    rs0 = sb.tile([64, 1], F32, tag="rs0")
    nc.vector.reciprocal(out=rs0, in_=sd0)
    nc.vector.tensor_scalar(out=xT.bitcast(F32R), in0=xT,
                            scalar1=mv0[:, 0:1], scalar2=rs0,
                            op0=AX.subtract, op1=AX.mult)

    hpT = sb.tile([128, 2, N], F32, tag="hpT")
    hpr = sb.tile([128, 2, 2, 65], F32, tag="hpr")
    ones4 = const.tile([128, 4], F32, tag="ones4")
    nc.vector.memset(ones4, 1.0)
    nc.vector.tensor_copy(out=hpr.bitcast(F32R)[:, :, :, 64], in_=ones4)
    E0 = sb.tile([128, 4, N], F32, tag="E0")
    u0a = sb.tile([65, 4 * N + 1024], F32, tag="u0a")
    nc.vector.memset(u0a, 0.0)

    d0d = dram.tile([1, 4 * N], F32, tag="d0d")
    D0b = sb.tile([64, 2 * N], F32, tag="D0b")
    x1T = sb.tile([64, 4, N], F32, tag="x1T")
    st1 = sb.tile([64, 4, 6], F32, tag="st1")
    mv1 = sb.tile([64, 4, 2], F32, tag="mv1")
    sd1 = sb.tile([64, 4], F32, tag="sd1")
    rs1 = sb.tile([64, 4], F32, tag="rs1")

    w1s = sb.tile([64, 4, 128], F32, tag="w1s")
    hpb = sb.tile([128, 1], F32, tag="hpb")
    hpT1 = sb.tile([128, N], F32, tag="hpT1")
    hpr1 = sb.tile([128, 2, 65], BF16, tag="hpr1")
    nc.vector.memset(hpr1[:, :, 64], 1.0)
    E1 = sb.tile([128, 2, N], BF16, tag="E1")
    u1a = sb.tile([65, N + 1024], F32, tag="u1a")
    nc.vector.memset(u1a, 0.0)

    # staging tiles for m-dependent stationary operands (ldweights cannot
    # take register offsets; DMA can)
    a0m = sb.tile([128, 2, 128], F32, tag="a0m")
    hpTm = sb.tile([128, 2, 128], F32, tag="hpTm")
    a1m = sb.tile([128, 256], F32, tag="a1m")
    hpT1m = sb.tile([128, 256], F32, tag="hpT1m")

    # ---- h'0 projection (static: hpT is loop-invariant) ----
    for j in range(2):
        for q in range(2):
            hp_ps = ps.tile([128, 1024], F32, tag="zf", bufs=2,
                            name="hp_ps")
            for c in range(2):
                off = q * 1024 + c * 512
                nc.tensor.matmul(
                    hp_ps[:, c * 512:(c + 1) * 512],
                    w0_sb[:, j, :].bitcast(F32R),
                    xT[:, off:off + 512].bitcast(F32R),
                    start=True, stop=True)
            dst = hpT[:, j, q * 1024:(q + 1) * 1024].bitcast(F32R)
            if q == 0:
                nc.scalar.copy(out=dst, in_=hp_ps)
            else:
                nc.vector.tensor_copy(out=dst, in_=hp_ps)

    hpT1_prev = sb.tile([128, N], F32, tag="hpT1p")

    # ---------------- the mega-loop ----------------
    with tc.For_i(0, N_ITER, 1) as i:
        m0c = (i % 16) * 128
        sel0 = 1 - (i // 16)
        mpc = ((i + 6) % 8) * 256
        sel1 = i // 18

        # ---- prefetch/stale reads first: all long-latency producers ----
        nc.sync.dma_start(out=a0m.bitcast(F32R),
                          in_=a0_d.ap()[:, :, ds(m0c, 128)].bitcast(F32R))
        nc.sync.dma_start(out=hpTm.bitcast(F32R),
                          in_=hpT[:, :, ds(m0c, 128)].bitcast(F32R))
        nc.sync.dma_start(out=a1m.bitcast(F32R),
                          in_=a1_d.ap()[:, ds(mpc, 256)].bitcast(F32R))
        # stale hpT1 (last iteration's glue): correct from i = 17
        nc.scalar.copy(out=hpT1_prev.bitcast(F32R), in_=hpT1)
        nc.sync.dma_start(out=hpT1m.bitcast(F32R),
                          in_=hpT1_prev[:, ds(mpc, 256)].bitcast(F32R))
        # stale denominator bounce (u0a row 64 is final from i = 16)
        nc.sync.dma_start(out=d0d, in_=u0a[64:65, 0:4 * N])
        # stale instance-norm1 stats of x1T (correct from i = 17)
        for h in range(4):
            for c in range(4):
                nc.vector.bn_stats(out=st1[:, c, :],
                                   in_=x1T[:, h, c * 512:(c + 1) * 512])
            nc.vector.bn_aggr(out=mv1[:, h, :], in_=st1)
        nc.scalar.activation(out=sd1, in_=mv1[:, :, 1], func=AF.Sqrt,
                             bias=eps_sb)
        nc.vector.reciprocal(out=rs1, in_=sd1)

        # ---- layer0 attention, m-tile i%16 ----
        tr_ps = ps.tile([128, 256], F32, tag="tr", bufs=1, name="tr_ps")
        for j in range(2):
            nc.tensor.transpose(tr_ps[:, j * 128:(j + 1) * 128],
                                hpTm[:, j, :], ident)
        nc.vector.tensor_copy(
            out=hpr.bitcast(F32R)[:, :, :, 0:64],
            in_=tr_ps.rearrange("p (j s o) -> p j s o", j=2, s=2))

        for j in range(2):
            for s in range(2):
                for q in range(2):
                    zt = ps.tile([128, 1024], F32, tag="zf", bufs=2,
                                 name="zt")
                    for c in range(2):
                        off = q * 1024 + c * 512
                        nc.tensor.matmul(
                            zt[:, c * 512:(c + 1) * 512],
                            a0m[64 * s:64 * s + 64, j, :].bitcast(F32R),
                            hpT[64 * s:64 * s + 64, j,
                                off:off + 512].bitcast(F32R),
                            start=True, stop=True)
                    nc.scalar.activation(
                        out=E0.bitcast(F32R)[:, 2 * j + s,
                                             q * 1024:(q + 1) * 1024],
                        in_=zt, func=AF.Prelu, alpha=NEG_SLOPE)
        nc.scalar.activation(out=E0.bitcast(F32R), in_=E0, func=AF.Exp)

        for j in range(2):
            for s in range(2):
                h = 2 * j + s
                for q in range(2):
                    u_ps = ps.tile([65, 1024], F32, tag="uf", bufs=1,
                                   name="u_ps")
                    for c in range(2):
                        off = q * 1024 + c * 512
                        nc.tensor.matmul(u_ps[:, c * 512:(c + 1) * 512],
                                         hpr[:, j, s, :].bitcast(F32R),
                                         E0[:, h,
                                            off:off + 512].bitcast(F32R),
                                         start=True, stop=True)
                    uoff = 4 * N - sel0 * (4 * N - (h * N + q * 1024))
                    nc.vector.tensor_tensor(out=u0a[:, ds(uoff, 1024)],
                                            in0=u0a[:, ds(uoff, 1024)],
                                            in1=u_ps, op=AX.add)

        # ---- glue (valid + idempotent from i = 16/17) ----
        for g in range(2):
            sl = slice(g * 2 * N, (g + 1) * 2 * N)
            bcg = bass.AP(tensor=d0d.tensor,
                          offset=d0d.offset + g * 2 * N,
                          ap=[[0, 64], [1, 2 * N]])
            nc.sync.dma_start(out=D0b, in_=bcg)
            nc.vector.reciprocal(out=D0b, in_=D0b)
            nc.vector.tensor_tensor(
                out=x1T.rearrange("p h n -> p (h n)").bitcast(F32R)[:, sl],
                in0=u0a[0:64, sl], in1=D0b, op=AX.mult)
        x1flat = x1T.rearrange("p h n -> p (h n)")
        nc.scalar.activation(out=x1flat.bitcast(F32R), in_=x1flat,
                             func=AF.Gelu, bias=b0_sb)

        # fold instance-norm1 into h'1: w1s = w1*rs (per channel), and
        # hpb[o'] = sum_c w1s[c,o']*mu_c subtracted at psum drain
        for kh in range(4):
            nc.vector.tensor_scalar(out=w1s.bitcast(F32R)[:, kh, :],
                                    in0=w1_sb[:, kh, :],
                                    scalar1=rs1[:, kh:kh + 1], scalar2=None,
                                    op0=AX.mult)
        hb_ps = ps.tile([128, 1], F32, tag="hb", bufs=1, name="hb_ps")
        for kh in range(4):
            nc.tensor.matmul(hb_ps, w1s[:, kh, :], mv1[:, kh, 0:1],
                             start=(kh == 0), stop=(kh == 3))
        nc.vector.tensor_copy(out=hpb, in_=hb_ps)

        for q in range(2):
            hp1_ps = ps.tile([128, 1024], F32, tag="zf", bufs=2,
                             name="hp1_ps")
            for c in range(2):
                for kh in range(4):
                    off = q * 1024 + c * 512
                    nc.tensor.matmul(
                        hp1_ps[:, c * 512:(c + 1) * 512],
                        w1s[:, kh, :].bitcast(F32R),
                        x1T[:, kh, off:off + 512].bitcast(F32R),
                        start=(kh == 0), stop=(kh == 3))
            dst1 = hpT1[:, q * 1024:(q + 1) * 1024].bitcast(F32R)
            nc.vector.tensor_scalar(out=dst1, in0=hp1_ps,
                                    scalar1=hpb, scalar2=None,
                                    op0=AX.subtract)

        # ---- layer1 attention, m-pair (i+6)%8 (stale h\'1) ----
        tr1_ps = ps.tile([128, 256], F32, tag="tr", bufs=1, name="tr1_ps")
        for s in range(2):
            nc.tensor.transpose(tr1_ps[:, s * 128:s * 128 + 64],
                                hpT1m[0:64, s * 128:(s + 1) * 128],
                                ident[0:64, 0:64])
        nc.vector.tensor_copy(
            out=hpr1[:, :, 0:64],
            in_=tr1_ps.rearrange("p (s o) -> p s o", s=2)[:, :, 0:64])

        for s in range(2):
            for q in range(2):
                zt1 = ps.tile([128, 1024], F32, tag="zf", bufs=2,
                              name="zt1")
                for c in range(2):
                    off = q * 1024 + c * 512
                    nc.tensor.matmul(
                        zt1[:, c * 512:(c + 1) * 512],
                        a1m[64 * s:64 * s + 64,
                            s * 128:(s + 1) * 128].bitcast(F32R),
                        hpT1_prev[64 * s:64 * s + 64,
                                  off:off + 512].bitcast(F32R),
                        start=True, stop=True)
                nc.scalar.activation(
                    out=E1[:, s, q * 1024:(q + 1) * 1024],
                    in_=zt1, func=AF.Prelu, alpha=NEG_SLOPE)
        nc.scalar.activation(out=E1, in_=E1, func=AF.Exp)

        for q in range(2):
            u1_ps = ps.tile([65, 1024], F32, tag="uf", bufs=1, name="u1_ps")
            for c in range(2):
                off = q * 1024 + c * 512
                for s in range(2):
                    nc.tensor.matmul(u1_ps[:, c * 512:(c + 1) * 512],
                                     hpr1[:, s, :],
                                     E1[:, s, off:off + 512],
                                     start=(s == 0), stop=(s == 1))
            u1off = N - sel1 * (N - q * 1024)
            nc.vector.tensor_tensor(out=u1a[:, ds(u1off, 1024)],
                                    in0=u1a[:, ds(u1off, 1024)],
                                    in1=u1_ps, op=AX.add)

    # ---------------- static epilogue ----------------
    d1d = dram.tile([1, N], F32, tag="d1d")
    nc.sync.dma_start(out=d1d, in_=u1a[64:65, 0:N])
    D1b = sb.tile([64, N], F32, tag="D1b")
    bc1 = bass.AP(tensor=d1d.tensor, offset=d1d.offset,
                  ap=[[0, 64], [1, N]])
    nc.sync.dma_start(out=D1b, in_=bc1)
    nc.vector.reciprocal(out=D1b, in_=D1b)
    outT = sb.tile([64, N], F32, tag="outT")
    nc.vector.tensor_tensor(out=outT, in0=u1a[0:64, 0:N], in1=D1b,
                            op=AX.mult)
    nc.sync.dma_start(out=out_d.ap().rearrange("n f -> f n"), in_=outT)
    if dbg:
        nc.sync.dma_start(out=dbg['hpT'].ap(), in_=hpT)
        nc.sync.dma_start(out=dbg['E0'].ap(), in_=E0)
        nc.sync.dma_start(out=dbg['u0a'].ap(), in_=u0a)
        nc.sync.dma_start(out=dbg['x1T'].ap(), in_=x1T)
        nc.sync.dma_start(out=dbg['hpT1'].ap(), in_=hpT1)
        nc.sync.dma_start(out=dbg['E1'].ap(), in_=E1)
        nc.sync.dma_start(out=dbg['u1a'].ap(), in_=u1a)


def _prep_host(inputs):
    f32 = np.float32
    asum0 = (np.asarray(inputs['a_src0'], f32)
             + np.asarray(inputs['a_dst0'], f32))        # [4, 64, n]
    a0 = np.empty((128, 2, N), f32)
    for h in range(4):
        a0[64 * (h % 2):64 * (h % 2) + 64, h // 2, :] = asum0[h]
    w0r = np.asarray(inputs['w0'], f32)                  # [4, 64, 64]
    w0 = np.empty((64, 2, 128), f32)
    for j in range(2):
        w0[:, j, 0:64] = w0r[2 * j]
        w0[:, j, 64:128] = w0r[2 * j + 1]
    b0 = np.ascontiguousarray(np.asarray(inputs['bias0'], f32).reshape(64, 1))
    asum1 = (np.asarray(inputs['a_src1'], f32)
             + np.asarray(inputs['a_dst1'], f32))[0]     # [64, n]
    a1 = np.concatenate([asum1, asum1], axis=0)          # [128, n]
    w1r = np.asarray(inputs['w1'], f32)[0].reshape(4, 64, 64)
    w1 = np.empty((64, 4, 128), f32)
    for kh in range(4):
        w1[:, kh, 0:64] = w1r[kh]
        w1[:, kh, 64:128] = w1r[kh]
    return {'a0': np.ascontiguousarray(a0),
            'w0': np.ascontiguousarray(w0), 'b0': b0,
            'a1': np.ascontiguousarray(a1),
            'w1': np.ascontiguousarray(w1)}


_NC_CACHE = {}


def _get_nc(sim_safe=False, repeat=1):
    # first positional arg kept for interface compatibility; unused.
    if repeat not in _NC_CACHE:
        _NC_CACHE[repeat] = build_bass(repeat=repeat)
    return _NC_CACHE[repeat]


def kernel(**inputs):
    from concourse.bass_utils import run_bass_kernel_spmd

    nc = _get_nc()
    w = _prep_host(inputs)
    x = np.asarray(inputs['x'], dtype=np.float32)
    in_maps = [{'x': np.ascontiguousarray(x[i]), **w} for i in range(N_CORES)]

    res = run_bass_kernel_spmd(nc, in_maps, core_ids=list(range(N_CORES)))
    out = np.stack([res.results[i]['out'] for i in range(N_CORES)])
    out = out + np.asarray(inputs['bias1'], dtype=np.float32)[None, None, :]
    return out.astype(np.float32)
